# Initial kernel scaffold
#
"""Mamba block (LN -> Mamba SSM -> residual -> LN -> FFN -> residual) on 8 NeuronCores.

Sharding: tensor-parallel over d_inner for in_proj/conv/scan/out_proj
(256 channels/core, full sequence -> scan is fully core-local);
sequence-parallel for LN1 and the FFN (512 token-rows/core).
Collectives: AllGather(ln1 out), AllReduce(x_proj partial), ReduceScatter(out_proj partial).
"""

import numpy as np
import ml_dtypes

import concourse.bass as bass
import concourse.mybir as mybir
import concourse.tile as tile
from concourse import bacc
from concourse.bass_utils import run_bass_kernel_spmd

BF16 = mybir.dt.bfloat16
F32 = mybir.dt.float32
AF = mybir.ActivationFunctionType
OP = mybir.AluOpType

B, L, DM = 2, 2048, 1024
DI, DS, DC, DTR, DFF = 2048, 16, 4, 64, 4096
NCORES = 8
CH = DI // NCORES          # 256 d_inner channels per core
TOK = B * L // NCORES      # 512 token-rows per core (256 per batch)
NT = B * L                 # 4096 total token rows
HL = L // 2                # scan half-tile length
GROUPS = [list(range(NCORES))]

_CACHE = {}


def bcast_rows(ap, p=128):
    """Partition-broadcast a row AP (free dim only) to [p, n] for DMA."""
    return bass.AP(tensor=ap.tensor, offset=ap.offset, ap=[[0, p], *ap.ap[-1:]])


def _ln(nc, wp, x_t, w_bc, b_bc, eps_t, out_bf):
    """Layernorm over free dim of x_t [128, D] (in place); writes bf16 out_bf."""
    D = x_t.shape[-1]
    stats = wp.tile([128, 2, 6], F32, tag="ln_stats")
    for s in range(2):
        nc.vector.bn_stats(out=stats[:, s, :], in_=x_t[:, s * (D // 2):(s + 1) * (D // 2)])
    mv = wp.tile([128, 2], F32, tag="ln_mv")
    nc.vector.bn_aggr(out=mv[:], in_=stats[:])
    rstd = wp.tile([128, 1], F32, tag="ln_rstd")
    nc.scalar.activation(out=rstd[:], in_=mv[:, 1:2], func=AF.Sqrt, bias=eps_t[:], scale=1.0)
    nc.vector.reciprocal(out=rstd[:], in_=rstd[:])
    nc.vector.tensor_scalar(out=x_t[:], in0=x_t[:], scalar1=mv[:, 0:1], scalar2=rstd[:],
                            op0=OP.subtract, op1=OP.mult)
    nc.vector.tensor_tensor(out=x_t[:], in0=x_t[:], in1=w_bc[:], op=OP.mult)
    nc.vector.tensor_tensor(out=out_bf[:], in0=x_t[:], in1=b_bc[:], op=OP.add)


def build():
    if "nc" in _CACHE:
        return _CACHE["nc"]
    nc = bacc.Bacc()

    # ---------------- I/O ----------------
    x_own = nc.declare_dram_parameter("x_own", [TOK, DM], F32, isOutput=False)
    w_in = nc.declare_dram_parameter("w_in", [128, 8, 512], BF16, isOutput=False)
    w_xp = nc.declare_dram_parameter("w_xp", [128, 2, 96], BF16, isOutput=False)
    w_dt = nc.declare_dram_parameter("w_dt", [128, 2, 128], BF16, isOutput=False)
    w_out = nc.declare_dram_parameter("w_out", [128, 2, 1024], BF16, isOutput=False)
    w1t = nc.declare_dram_parameter("w1t", [32, 128, 8, 128], BF16, isOutput=False)
    w2t = nc.declare_dram_parameter("w2t", [DFF, DM], BF16, isOutput=False)
    conv_w = nc.declare_dram_parameter("conv_w", [128, 2, 4], F32, isOutput=False)
    conv_b = nc.declare_dram_parameter("conv_b", [128, 2], F32, isOutput=False)
    dt_b = nc.declare_dram_parameter("dt_b", [128, 2], F32, isOutput=False)
    a_mat = nc.declare_dram_parameter("a_mat", [128, 2, DS], F32, isOutput=False)
    dp_vec = nc.declare_dram_parameter("dp_vec", [128, 2], F32, isOutput=False)
    ln1w = nc.declare_dram_parameter("ln1w", [1, DM], BF16, isOutput=False)
    ln1b = nc.declare_dram_parameter("ln1b", [1, DM], BF16, isOutput=False)
    ln2w = nc.declare_dram_parameter("ln2w", [1, DM], BF16, isOutput=False)
    ln2b = nc.declare_dram_parameter("ln2b", [1, DM], BF16, isOutput=False)
    b1m = nc.declare_dram_parameter("b1m", [128, 32], F32, isOutput=False)
    b2v = nc.declare_dram_parameter("b2v", [1, DM], BF16, isOutput=False)
    out_p = nc.declare_dram_parameter("out", [TOK, DM], F32, isOutput=True)

    # ---------------- internal DRAM ----------------
    xn_own_d = nc.dram_tensor("xn_own_d", [TOK, DM], BF16)
    xn_all_d = nc.dram_tensor("xn_all_d", [NT, DM], BF16, addr_space="Shared")
    xdbl_part = nc.dram_tensor("xdbl_part", [96, NT], F32)
    xdbl_full = nc.dram_tensor("xdbl_full", [96, NT], F32, addr_space="Shared")
    bc_d = nc.dram_tensor("bc_d", [32, NT], BF16)
    z_d = nc.dram_tensor("z_d", [128, 2 * NT], BF16)
    yp_part = nc.dram_tensor("yp_part", [NT, DM], BF16)
    mamba_own_d = nc.dram_tensor("mamba_own_d", [TOK, DM], BF16)
    xn2_d = nc.dram_tensor("xn2_d", [TOK, DM], BF16)

    with tile.TileContext(nc) as tc:
        with (
            tc.tile_pool(name="const", bufs=1) as cp,
            tc.tile_pool(name="xp", bufs=2) as xpool,
            tc.tile_pool(name="work", bufs=2) as wp,
            tc.tile_pool(name="mid", bufs=2) as mp,
            tc.tile_pool(name="scan", bufs=2) as sp,
            tc.tile_pool(name="hpool", bufs=2) as hp,
            tc.tile_pool(name="acc", bufs=1) as accp,
            tc.tile_pool(name="psum", bufs=4, space="PSUM") as pp,
            tc.tile_pool(name="psum2", bufs=2, space="PSUM") as pp2,
        ):
            # ---- resident constants ----
            w_in_s = cp.tile([128, 8, 512], BF16, tag="w_in")
            nc.sync.dma_start(out=w_in_s[:], in_=w_in[:, :, :])
            w_xp_s = cp.tile([128, 2, 96], BF16, tag="w_xp")
            nc.sync.dma_start(out=w_xp_s[:], in_=w_xp[:, :, :])
            w_dt_s = cp.tile([128, 2, 128], BF16, tag="w_dt")
            nc.sync.dma_start(out=w_dt_s[:], in_=w_dt[:, :, :])
            w_out_s = cp.tile([128, 2, 1024], BF16, tag="w_out")
            nc.sync.dma_start(out=w_out_s[:], in_=w_out[:, :, :])
            conv_w_s = cp.tile([128, 2, 4], F32, tag="conv_w")
            nc.sync.dma_start(out=conv_w_s[:], in_=conv_w[:, :, :])
            conv_b_s = cp.tile([128, 2], F32, tag="conv_b")
            nc.sync.dma_start(out=conv_b_s[:], in_=conv_b[:, :])
            dt_b_s = cp.tile([128, 2], F32, tag="dt_b")
            nc.sync.dma_start(out=dt_b_s[:], in_=dt_b[:, :])
            a_s = cp.tile([128, 2, DS], F32, tag="a_mat")
            nc.sync.dma_start(out=a_s[:], in_=a_mat[:, :, :])
            dp_s = cp.tile([128, 2], F32, tag="dp")
            nc.sync.dma_start(out=dp_s[:], in_=dp_vec[:, :])
            b1_s = cp.tile([128, 32], F32, tag="b1")
            nc.sync.dma_start(out=b1_s[:], in_=b1m[:, :])
            ln1w_s = cp.tile([128, DM], BF16, tag="ln1w")
            nc.sync.dma_start(out=ln1w_s[:], in_=bcast_rows(ln1w[0, :]))
            ln1b_s = cp.tile([128, DM], BF16, tag="ln1b")
            nc.sync.dma_start(out=ln1b_s[:], in_=bcast_rows(ln1b[0, :]))
            ln2w_s = cp.tile([128, DM], BF16, tag="ln2w")
            nc.sync.dma_start(out=ln2w_s[:], in_=bcast_rows(ln2w[0, :]))
            ln2b_s = cp.tile([128, DM], BF16, tag="ln2b")
            nc.sync.dma_start(out=ln2b_s[:], in_=bcast_rows(ln2b[0, :]))
            b2_s = cp.tile([128, DM], BF16, tag="b2")
            nc.sync.dma_start(out=b2_s[:], in_=bcast_rows(b2v[0, :]))
            eps_s = cp.tile([128, 1], F32, tag="eps")
            nc.vector.memset(eps_s[:], 1e-5)
            ones_s = cp.tile([1, 128], BF16, tag="ones")
            nc.vector.memset(ones_s[:], 1.0)

            # ---- resident activations ----
            u_raw = cp.tile([128, 2, NT], BF16, tag="u_raw")   # ch-chunk x (b,t); later y
            delta = cp.tile([128, 2, NT], BF16, tag="big_a")   # slot reused by ffn h1
            du = cp.tile([128, 2, NT], BF16, tag="dux2")       # slot reused by x2

            # ================= Phase 1: LN1 (own tokens) + AllGather ================
            for tt in range(4):
                x_t = wp.tile([128, DM], F32, tag="x_in")
                nc.sync.dma_start(out=x_t[:], in_=x_own[tt * 128:(tt + 1) * 128, :])
                xn_bf = wp.tile([128, DM], BF16, tag="xn_bf")
                _ln(nc, wp, x_t, ln1w_s, ln1b_s, eps_s, xn_bf)
                nc.sync.dma_start(out=xn_own_d[tt * 128:(tt + 1) * 128, :], in_=xn_bf[:])
            nc.gpsimd.collective_compute(
                "AllGather", OP.bypass, replica_groups=GROUPS,
                ins=[xn_own_d[:, :]], outs=[xn_all_d[:, :]])

            # ================= Phase 2: in_proj (channel-major out) =================
            # xn_all rows are ordered (core, b, t_local): row = c*512 + b*256 + tl.
            for b in range(B):
                for tc4 in range(4):          # tok-chunks of 512 within L
                    g0 = b * L + tc4 * 512
                    xnT = xpool.tile([128, 2, 8, 256], BF16, tag="xnT")
                    for h in range(2):        # two 256-row spans (two source cores)
                        c_src = 2 * tc4 + h
                        r0 = c_src * 512 + b * 256
                        nc.sync.dma_start_transpose(
                            xnT[:, h, :, :], xn_all_d[r0:r0 + 256, :])
                    psums = [pp.tile([128, 512], F32, tag="ps", name=f"ps_in{m}")
                             for m in range(4)]
                    for h in range(2):
                        for k in range(8):
                            for m in range(4):
                                nc.tensor.matmul(
                                    psums[m][:, h * 256:(h + 1) * 256],
                                    w_in_s[:, k, m * 128:(m + 1) * 128],
                                    xnT[:, h, k, :], start=(k == 0), stop=(k == 7))
                    for m in range(2):        # u chunks stay in SBUF
                        nc.scalar.copy(out=u_raw[:, m, g0:g0 + 512], in_=psums[m][:])
                    for m in range(2, 4):     # z chunks spill to DRAM
                        zt = wp.tile([128, 512], BF16, tag="zt")
                        nc.scalar.copy(out=zt[:], in_=psums[m][:])
                        nc.gpsimd.dma_start(
                            out=z_d[:, (m - 2) * NT + g0:(m - 2) * NT + g0 + 512],
                            in_=zt[:])

            # ================= Phase 3: conv + silu (per ch-chunk, per batch) =======
            for cc in range(2):
                for b in range(B):
                    g0 = b * L
                    u_sl = u_raw[:, cc, g0:g0 + L]
                    cv = mp.tile([128, L], F32, tag="t8")
                    nc.vector.tensor_scalar(
                        out=cv[:], in0=u_sl, scalar1=conv_w_s[:, cc, 3:4],
                        scalar2=conv_b_s[:, cc:cc + 1], op0=OP.mult, op1=OP.add)
                    for k, sh in ((2, 1), (1, 2), (0, 3)):
                        nc.vector.scalar_tensor_tensor(
                            out=cv[:, sh:], in0=u_raw[:, cc, g0:g0 + L - sh],
                            scalar=conv_w_s[:, cc, k:k + 1], in1=cv[:, sh:],
                            op0=OP.mult, op1=OP.add)
                    nc.scalar.activation(out=u_sl, in_=cv[:], func=AF.Silu)

            # ================= Phase 4: x_proj partial + AllReduce ==================
            for b in range(B):
                for tc4 in range(4):
                    g0 = b * L + tc4 * 512
                    ps = pp.tile([128, 512], F32, tag="ps")
                    for cc in range(2):
                        nc.tensor.matmul(ps[:96, :], w_xp_s[:, cc, :],
                                         u_raw[:, cc, g0:g0 + 512],
                                         start=(cc == 0), stop=(cc == 1))
                    xd_sb = wp.tile([96, 512], F32, tag="xd_sb")
                    nc.scalar.copy(out=xd_sb[:], in_=ps[:96, :])
                    nc.sync.dma_start(out=xdbl_part[:, g0:g0 + 512], in_=xd_sb[:])
            nc.gpsimd.collective_compute(
                "AllReduce", OP.add, replica_groups=GROUPS,
                ins=[xdbl_part[:, :]], outs=[xdbl_full[:, :]])

            # ---- stage B/C rows as bf16 in DRAM for partition-broadcast ----
            for ch2 in range(2):
                c0 = ch2 * 2048
                xs = wp.tile([32, 2048], F32, tag="xs", name=f"xs{ch2}")
                nc.sync.dma_start(out=xs[:], in_=xdbl_full[64:96, c0:c0 + 2048])
                bcc = wp.tile([32, 2048], BF16, tag="bcc", name=f"bcc{ch2}")
                nc.scalar.copy(out=bcc[:], in_=xs[:])
                nc.sync.dma_start(out=bc_d[:, c0:c0 + 2048], in_=bcc[:])

            # ================= Phase 5: dt_proj + softplus -> delta =================
            for tc8 in range(8):
                g0 = tc8 * 512
                dt_f = wp.tile([64, 512], F32, tag="dt_f")
                nc.sync.dma_start(out=dt_f[:], in_=xdbl_full[:64, g0:g0 + 512])
                dt_bft = wp.tile([64, 512], BF16, tag="dt_bft")
                nc.scalar.copy(out=dt_bft[:], in_=dt_f[:])
                for cc in range(2):
                    ps = pp.tile([128, 512], F32, tag="ps")
                    nc.tensor.matmul(ps[:], w_dt_s[:64, cc, :], dt_bft[:],
                                     start=True, stop=True)
                    # softplus(z) = ln(1 + exp(z)); ln/exp share one ACT table
                    ex = wp.tile([128, 512], F32, tag="sp_ex")
                    nc.scalar.activation(out=ex[:], in_=ps[:], func=AF.Exp,
                                         bias=dt_b_s[:, cc:cc + 1], scale=1.0)
                    nc.scalar.activation(out=delta[:, cc, g0:g0 + 512], in_=ex[:],
                                         func=AF.Ln, bias=1.0, scale=1.0)
            for cc in range(2):
                nc.vector.tensor_tensor(out=du[:, cc, :], in0=delta[:, cc, :],
                                        in1=u_raw[:, cc, :], op=OP.mult)

            # ================= Phase 6: selective scan ==============================
            for b in range(B):
                for cc in range(2):
                    g0 = b * L
                    y_acc = accp.tile([128, L], F32, tag="y_acc")
                    nc.vector.tensor_scalar_mul(out=y_acc[:], in0=u_raw[:, cc, g0:g0 + L],
                                                scalar1=dp_s[:, cc:cc + 1])
                    for n in range(DS):
                        br = sp.tile([1, L], BF16, tag="br")
                        nc.gpsimd.dma_start(out=br[:], in_=bc_d[n:n + 1, g0:g0 + L])
                        cr = sp.tile([1, L], BF16, tag="cr")
                        nc.gpsimd.dma_start(out=cr[:], in_=bc_d[16 + n:17 + n, g0:g0 + L])
                        h_prev = None
                        for hh in range(2):   # half-tiles along L, chained via initial
                            hg = g0 + hh * HL
                            b_ps = pp2.tile([128, HL], F32, tag="bc_ps", name="b_ps")
                            c_ps = pp2.tile([128, HL], F32, tag="bc_ps", name="c_ps")
                            for q in range(HL // 512):
                                sl = slice(hh * HL + q * 512, hh * HL + (q + 1) * 512)
                                qs = slice(q * 512, (q + 1) * 512)
                                nc.tensor.matmul(b_ps[:, qs], ones_s[:], br[:, sl],
                                                 start=True, stop=True)
                                nc.tensor.matmul(c_ps[:, qs], ones_s[:], cr[:, sl],
                                                 start=True, stop=True)
                            a_n = sp.tile([128, HL], BF16, tag="a_n")
                            nc.scalar.activation(out=a_n[:], in_=delta[:, cc, hg:hg + HL],
                                                 func=AF.Exp, scale=a_s[:, cc, n:n + 1])
                            b_bc = sp.tile([128, HL], BF16, tag="b_bc")
                            nc.vector.tensor_tensor(out=b_bc[:],
                                                    in0=du[:, cc, hg:hg + HL],
                                                    in1=b_ps[:], op=OP.mult)
                            h_n = hp.tile([128, HL], BF16, tag="h_n")
                            nc.vector.tensor_tensor_scan(
                                out=h_n[:], data0=a_n[:], data1=b_bc[:],
                                initial=(0.0 if hh == 0 else h_prev[:, 0:1]),
                                op0=OP.mult, op1=OP.add)
                            if hh == 0:
                                h_last = hp.tile([128, 1], F32, tag="h_last")
                                nc.vector.tensor_copy(out=h_last[:], in_=h_n[:, HL - 1:HL])
                                h_prev = h_last
                            nc.vector.tensor_tensor(out=h_n[:], in0=h_n[:], in1=c_ps[:],
                                                    op=OP.mult)
                            add_eng = nc.gpsimd if n % 2 == 0 else nc.vector
                            add_eng.tensor_tensor(out=y_acc[:, hh * HL:(hh + 1) * HL],
                                                  in0=y_acc[:, hh * HL:(hh + 1) * HL],
                                                  in1=h_n[:], op=OP.add)
                    # gate: u_raw slice <- y_acc * silu(z)
                    zf = mp.tile([128, L], BF16, tag="t8")
                    nc.gpsimd.dma_start(out=zf[:], in_=z_d[:, cc * NT + g0:cc * NT + g0 + L])
                    nc.scalar.activation(out=zf[:], in_=zf[:], func=AF.Silu)
                    nc.vector.tensor_tensor(out=u_raw[:, cc, g0:g0 + L], in0=y_acc[:],
                                            in1=zf[:], op=OP.mult)

            # ================= Phase 7: out_proj -> ReduceScatter ===================
            # partial rows written in receiver-slot order:
            # token (b, t) -> slot row rc*512 + b*256 + (t - rc*256), rc = t // 256.
            for b in range(B):
                for tt in range(16):          # 128-token tiles within L
                    t0 = tt * 128
                    g0 = b * L + t0
                    rc = t0 // 256
                    r0 = rc * 512 + b * 256 + (t0 - rc * 256)
                    yp_sb = wp.tile([128, DM], BF16, tag="yp_sb")
                    for f in range(2):
                        ps = pp.tile([128, 512], F32, tag="ps")
                        for cc in range(2):
                            nc.tensor.matmul(
                                ps[:], u_raw[:, cc, g0:g0 + 128],
                                w_out_s[:, cc, f * 512:(f + 1) * 512],
                                start=(cc == 0), stop=(cc == 1))
                        nc.scalar.copy(out=yp_sb[:, f * 512:(f + 1) * 512], in_=ps[:])
                    nc.gpsimd.dma_start(out=yp_part[r0:r0 + 128, :], in_=yp_sb[:])
            nc.gpsimd.collective_compute(
                "ReduceScatter", OP.add, replica_groups=GROUPS,
                ins=[yp_part[:, :]], outs=[mamba_own_d[:, :]])

            # ================= Phase 8: residual + LN2 + FFN (own tokens) ===========
            x2 = cp.tile([128, 4, DM], F32, tag="dux2")
            for tt in range(4):
                nc.sync.dma_start(out=x2[:, tt, :], in_=x_own[tt * 128:(tt + 1) * 128, :])
                mo = wp.tile([128, DM], BF16, tag="mo")
                nc.sync.dma_start(out=mo[:], in_=mamba_own_d[tt * 128:(tt + 1) * 128, :])
                nc.vector.tensor_tensor(out=x2[:, tt, :], in0=x2[:, tt, :], in1=mo[:],
                                        op=OP.add)
                xc = wp.tile([128, DM], F32, tag="x_in")
                nc.vector.tensor_copy(out=xc[:], in_=x2[:, tt, :])
                xn2_bf = wp.tile([128, DM], BF16, tag="xn_bf")
                _ln(nc, wp, xc, ln2w_s, ln2b_s, eps_s, xn2_bf)
                nc.sync.dma_start(out=xn2_d[tt * 128:(tt + 1) * 128, :], in_=xn2_bf[:])

            xn2T = cp.tile([128, 8, 512], BF16, tag="w_in")   # reuses w_in slot
            for k in range(8):
                nc.sync.dma_start_transpose(
                    xn2T[:, k, :], xn2_d[:, k * 128:(k + 1) * 128])

            for half in range(2):             # 256-token halves to halve h1 footprint
                ts0 = half * 256
                h1 = cp.tile([128, 32, 256], BF16, tag="big_a", name=f"h1_{half}")
                for m in range(32):
                    w1_t = wp.tile([128, 8, 128], BF16, tag="w1_t")
                    nc.sync.dma_start(out=w1_t[:], in_=w1t[m, :, :, :])
                    ps = pp.tile([128, 512], F32, tag="ps")
                    for k in range(8):
                        nc.tensor.matmul(ps[:, :256], w1_t[:, k, :],
                                         xn2T[:, k, ts0:ts0 + 256],
                                         start=(k == 0), stop=(k == 7))
                    nc.scalar.activation(out=h1[:, m, :], in_=ps[:, :256], func=AF.Relu,
                                         bias=b1_s[:, m:m + 1], scale=1.0)
                ps_o = [[pp.tile([128, 512], F32, tag="ps", name=f"ps_o{half}_{t_}_{f_}")
                         for f_ in range(2)] for t_ in range(2)]
                for k in range(32):
                    w2_t = wp.tile([128, DM], BF16, tag="w2_t")
                    nc.sync.dma_start(out=w2_t[:], in_=w2t[k * 128:(k + 1) * 128, :])
                    for t_ in range(2):
                        for f_ in range(2):
                            nc.tensor.matmul(
                                ps_o[t_][f_][:], h1[:, k, t_ * 128:(t_ + 1) * 128],
                                w2_t[:, f_ * 512:(f_ + 1) * 512],
                                start=(k == 0), stop=(k == 31))
                for t_ in range(2):
                    tt = half * 2 + t_
                    o_sb = mp.tile([128, DM], F32, tag="t8")
                    for f_ in range(2):
                        nc.vector.tensor_tensor(
                            out=o_sb[:, f_ * 512:(f_ + 1) * 512], in0=ps_o[t_][f_][:],
                            in1=x2[:, tt, f_ * 512:(f_ + 1) * 512], op=OP.add)
                    nc.vector.tensor_tensor(out=o_sb[:], in0=o_sb[:], in1=b2_s[:],
                                            op=OP.add)
                    nc.sync.dma_start(out=out_p[tt * 128:(tt + 1) * 128, :], in_=o_sb[:])

    nc.finalize()
    _CACHE["nc"] = nc
    return nc


def _prep_inputs(inputs):
    bf = ml_dtypes.bfloat16
    x = np.asarray(inputs["x"], np.float32)
    in_w = np.asarray(inputs["in_proj_w"], np.float32)
    conv_w = np.asarray(inputs["conv_w"], np.float32).reshape(DI, DC)
    conv_b = np.asarray(inputs["conv_b"], np.float32)
    xp_w = np.asarray(inputs["x_proj_w"], np.float32)
    dt_w = np.asarray(inputs["dt_proj_w"], np.float32)
    dt_bias = np.asarray(inputs["dt_proj_b"], np.float32)
    A = -np.exp(np.asarray(inputs["A_log"], np.float32))
    Dp = np.asarray(inputs["Dp"], np.float32)
    out_w = np.asarray(inputs["out_proj_w"], np.float32)
    w1 = np.asarray(inputs["ffn_w1"], np.float32)
    b1 = np.asarray(inputs["ffn_b1"], np.float32)
    w2 = np.asarray(inputs["ffn_w2"], np.float32)
    b2 = np.asarray(inputs["ffn_b2"], np.float32)

    common = {
        "w1t": np.ascontiguousarray(
            w1.T.reshape(8, 128, 32, 128).transpose(2, 1, 0, 3)).astype(bf),
        "w2t": np.ascontiguousarray(w2.T).astype(bf),
        "b1m": np.ascontiguousarray(b1.reshape(32, 128).T).astype(np.float32),
        "b2v": b2.reshape(1, DM).astype(bf),
        "ln1w": inputs["ln1_w"].reshape(1, DM).astype(bf),
        "ln1b": inputs["ln1_b"].reshape(1, DM).astype(bf),
        "ln2w": inputs["ln2_w"].reshape(1, DM).astype(bf),
        "ln2b": inputs["ln2_b"].reshape(1, DM).astype(bf),
    }

    def stripe2(v):  # [256] -> [128, 2]
        return np.ascontiguousarray(v.reshape(2, 128).T).astype(np.float32)

    in_maps = []
    for c in range(NCORES):
        ch0 = c * CH
        sl = slice(ch0, ch0 + CH)
        x_own = np.concatenate([x[b, c * (L // NCORES):(c + 1) * (L // NCORES), :]
                                for b in range(B)], axis=0)
        w_slice = np.concatenate([in_w[sl, :], in_w[DI + ch0:DI + ch0 + CH, :]], axis=0)
        w_in_t = np.ascontiguousarray(
            w_slice.T.reshape(8, 128, 512).transpose(1, 0, 2)).astype(bf)
        w_xp_t = np.ascontiguousarray(
            xp_w[:, sl].T.reshape(2, 128, 96).transpose(1, 0, 2)).astype(bf)
        dt_t = np.zeros((128, 256), np.float32)
        dt_t[:64, :] = dt_w[sl, :].T
        w_dt_t = np.ascontiguousarray(dt_t.reshape(128, 2, 128)).astype(bf)
        w_out_t = np.ascontiguousarray(
            out_w[:, sl].T.reshape(2, 128, 1024).transpose(1, 0, 2)).astype(bf)
        a_own = np.ascontiguousarray(
            A[sl].reshape(2, 128, DS).transpose(1, 0, 2)).astype(np.float32)
        cw = np.ascontiguousarray(
            conv_w[sl].reshape(2, 128, DC).transpose(1, 0, 2)).astype(np.float32)
        in_maps.append(dict(common,
            x_own=np.ascontiguousarray(x_own),
            w_in=w_in_t, w_xp=w_xp_t, w_dt=w_dt_t, w_out=w_out_t,
            conv_w=cw, conv_b=stripe2(conv_b[sl]), dt_b=stripe2(dt_bias[sl]),
            a_mat=a_own, dp_vec=stripe2(Dp[sl]),
        ))
    return in_maps


def _run(inputs, trace):
    nc = build()
    in_maps = _prep_inputs(inputs)
    res = run_bass_kernel_spmd(nc, in_maps, core_ids=list(range(NCORES)), trace=trace)
    out = np.empty((B, L, DM), np.float32)
    lc = L // NCORES
    for c in range(NCORES):
        o = res.results[c]["out"]
        for b in range(B):
            out[b, c * lc:(c + 1) * lc, :] = o[b * lc:(b + 1) * lc, :]
    return out, res


def kernel(**inputs):
    return _run(inputs, trace=False)[0]


def kernel_timed(**inputs):
    out, res = _run(inputs, trace=True)
    return out, res.exec_time_ns



# revision 4
# speedup vs baseline: 1.0162x; 1.0162x over previous
"""Mamba block (LN -> Mamba SSM -> residual -> LN -> FFN -> residual) on 8 NeuronCores.

v2: tensor-parallel over d_inner for the SSM (256 ch/core, full sequence),
sequence-parallel LN1/FFN. Per-batch split collectives for pipelining:
AllGather(ln1^T out), AllReduce(x_proj partial), ReduceScatter(out_proj).

Key changes vs v1: transposed LN1 (stats via ones-matmul, no DMA transposes),
decay powers via exp ladder (a_n = E^(n+1), E = sigmoid(-(dt_pre+bias))),
B/C broadcast to SBUF bf16 (ones-matmul + ACT copy) so the scan-side
elementwise ops run in the DVE 2x mode, y-accumulation as two running sums
(Pool chain seeded with Dp*u, DVE chain), FFN(b0) interleaved into scan(b1).
"""

import numpy as np
import ml_dtypes

import concourse.bass as bass
import concourse.mybir as mybir
import concourse.tile as tile
from concourse import bacc
from concourse.bass_utils import run_bass_kernel_spmd

BF16 = mybir.dt.bfloat16
F32 = mybir.dt.float32
AF = mybir.ActivationFunctionType
OP = mybir.AluOpType

B, L, DM = 2, 2048, 1024
DI, DS, DC, DTR, DFF = 2048, 16, 4, 64, 4096
NCORES = 8
CH = DI // NCORES          # 256 d_inner channels per core (2 chunks of 128)
TOK = B * L // NCORES      # 512 token-rows per core (256 per batch)
TB = TOK // B              # 256 own tokens per batch
NT = B * L                 # 4096 total token rows
GROUPS = [list(range(NCORES))]

_CACHE = {}


def build(ladder: bool):
    key = ("nc", ladder)
    if key in _CACHE:
        return _CACHE[key]
    nc = bacc.Bacc()

    # ---------------- I/O ----------------
    xT_own = nc.declare_dram_parameter("xT_own", [DM, TOK], F32, isOutput=False)
    x_own = nc.declare_dram_parameter("x_own", [TOK, DM], F32, isOutput=False)
    w_in = nc.declare_dram_parameter("w_in", [128, 8, 512], BF16, isOutput=False)
    w_xp = nc.declare_dram_parameter("w_xp", [128, 2, 96], BF16, isOutput=False)
    w_dt = nc.declare_dram_parameter("w_dt", [64, 2, 128], BF16, isOutput=False)
    w_out = nc.declare_dram_parameter("w_out", [128, 2, 1024], BF16, isOutput=False)
    w1t = nc.declare_dram_parameter("w1t", [32, 128, 8, 128], BF16, isOutput=False)
    w2t = nc.declare_dram_parameter("w2t", [DFF, DM], BF16, isOutput=False)
    conv_w = nc.declare_dram_parameter("conv_w", [128, 2, 4], F32, isOutput=False)
    conv_b = nc.declare_dram_parameter("conv_b", [128, 2], F32, isOutput=False)
    ndt_b = nc.declare_dram_parameter("ndt_b", [128, 2], F32, isOutput=False)
    dt_b = nc.declare_dram_parameter("dt_b", [128, 2], F32, isOutput=False)
    a_mat = nc.declare_dram_parameter("a_mat", [128, 2, DS], F32, isOutput=False)
    dp_vec = nc.declare_dram_parameter("dp_vec", [128, 2], F32, isOutput=False)
    ln1wT = nc.declare_dram_parameter("ln1wT", [128, 8], F32, isOutput=False)
    ln1bT = nc.declare_dram_parameter("ln1bT", [128, 8], F32, isOutput=False)
    ln2w = nc.declare_dram_parameter("ln2w", [1, DM], BF16, isOutput=False)
    ln2b = nc.declare_dram_parameter("ln2b", [1, DM], BF16, isOutput=False)
    b1m = nc.declare_dram_parameter("b1m", [128, 32], F32, isOutput=False)
    b2v = nc.declare_dram_parameter("b2v", [1, DM], BF16, isOutput=False)
    ident = nc.declare_dram_parameter("ident", [128, 128], BF16, isOutput=False)
    FP8 = mybir.dt.float8e4
    sel = nc.declare_dram_parameter("sel", [32, 32 * 128], FP8, isOutput=False)
    out_p = nc.declare_dram_parameter("out", [TOK, DM], F32, isOutput=True)

    # ---------------- internal DRAM ----------------
    xnT_b = [nc.dram_tensor(f"xnT_b{b}", [128, 8, TB], BF16) for b in range(B)]
    xnT_all = [nc.dram_tensor(f"xnT_all{b}", [NCORES * 128, 8, TB], BF16,
                              addr_space="Shared") for b in range(B)]
    xdbl_pt = [nc.dram_tensor(f"xdbl_pt{b}", [96, L], F32) for b in range(B)]
    xdbl_fl = [nc.dram_tensor(f"xdbl_fl{b}", [96, L], F32,
                              addr_space="Shared") for b in range(B)]
    z_d = nc.dram_tensor("z_d", [128, 2, NT], BF16)
    yp_b = [nc.dram_tensor(f"yp_b{b}", [L, DM], BF16) for b in range(B)]
    mamba_b = [nc.dram_tensor(f"mamba_b{b}", [TB, DM], BF16) for b in range(B)]

    from contextlib import ExitStack
    with tile.TileContext(nc) as tc, ExitStack() as est:
        pool = lambda *a, **k: est.enter_context(tc.tile_pool(*a, **k))
        cp = pool(name="const", bufs=1)
        xbp = pool(name="xbp", bufs=3)       # LN1 x chunks
        lnp = pool(name="lnp", bufs=3)       # LN1 smalls + bcasts
        inp = pool(name="inp", bufs=2)       # in_proj rhs tiles
        stg = pool(name="stg", bufs=2)       # small staging
        rp = pool(name="rows", bufs=1)       # dt rows per b
        bcp = pool(name="bc", bufs=3)        # broadcast tiles
        lp = pool(name="lad", bufs=4)        # ladder a_n tiles
        bbp = pool(name="bbp", bufs=2)
        hp = pool(name="hp", bufs=1)
        ppr = pool(name="ppr", bufs=2)       # products
        trp = pool(name="tr", bufs=2)        # tree accumulators
        cvp = pool(name="cv", bufs=2)        # conv / z staging
        x2p = pool(name="x2p", bufs=2)       # residual f32
        fwp = pool(name="ffw", bufs=2)       # ffn weight stream
        fhp = pool(name="ffh", bufs=2)       # ffn h1 tiles
        fxp = pool(name="ffx", bufs=1)       # xn2T per b
        obp = pool(name="ob", bufs=1)        # out staging
        p5 = pool(name="ps5", bufs=2, space="PSUM")
        ptr = pool(name="pstr", bufs=1, space="PSUM")
        p10 = pool(name="p10", bufs=4, space="PSUM")
        if True:
            # ---- resident constants ----
            w_in_s = cp.tile([128, 8, 512], BF16, tag="w_in")
            nc.sync.dma_start(out=w_in_s[:], in_=w_in[:, :, :])
            w_xp_s = cp.tile([128, 2, 96], BF16, tag="w_xp")
            nc.sync.dma_start(out=w_xp_s[:], in_=w_xp[:, :, :])
            w_dt_s = cp.tile([64, 2, 128], BF16, tag="w_dt")
            nc.sync.dma_start(out=w_dt_s[:], in_=w_dt[:, :, :])
            w_out_s = cp.tile([128, 2, 1024], BF16, tag="w_out")
            nc.sync.dma_start(out=w_out_s[:], in_=w_out[:, :, :])
            conv_w_s = cp.tile([128, 2, 4], F32, tag="conv_w")
            nc.sync.dma_start(out=conv_w_s[:], in_=conv_w[:, :, :])
            conv_b_s = cp.tile([128, 2], F32, tag="conv_b")
            nc.sync.dma_start(out=conv_b_s[:], in_=conv_b[:, :])
            ndt_b_s = cp.tile([128, 2], F32, tag="ndt_b")
            nc.sync.dma_start(out=ndt_b_s[:], in_=ndt_b[:, :])
            dt_b_s = cp.tile([128, 2], F32, tag="dt_b")
            nc.sync.dma_start(out=dt_b_s[:], in_=dt_b[:, :])
            a_s = cp.tile([128, 2, DS], F32, tag="a_mat")
            nc.sync.dma_start(out=a_s[:], in_=a_mat[:, :, :])
            dp_s = cp.tile([128, 2], F32, tag="dp")
            nc.sync.dma_start(out=dp_s[:], in_=dp_vec[:, :])
            ln1wT_s = cp.tile([128, 8], F32, tag="ln1wT")
            nc.sync.dma_start(out=ln1wT_s[:], in_=ln1wT[:, :])
            ln1bT_s = cp.tile([128, 8], F32, tag="ln1bT")
            nc.sync.dma_start(out=ln1bT_s[:], in_=ln1bT[:, :])
            b1_s = cp.tile([128, 32], F32, tag="b1")
            nc.sync.dma_start(out=b1_s[:], in_=b1m[:, :])
            ident_s = cp.tile([128, 128], BF16, tag="ident")
            nc.sync.dma_start(out=ident_s[:], in_=ident[:, :])
            sel_s = cp.tile([32, 32 * 128], FP8, tag="sel")
            nc.sync.dma_start(out=sel_s[:], in_=sel[:, :])
            ones1 = cp.tile([1, 128], BF16, tag="ones1")
            nc.vector.memset(ones1[:], 1.0)
            ones_col = cp.tile([128, 1], BF16, tag="ones_col")
            nc.vector.memset(ones_col[:], 1.0)
            eps_s = cp.tile([128, 1], F32, tag="eps")
            nc.vector.memset(eps_s[:], 1e-5)
            ln2w_row = cp.tile([1, DM], BF16, tag="ln2w_row")
            nc.sync.dma_start(out=ln2w_row[:], in_=ln2w[0:1, :])
            ln2b_row = cp.tile([1, DM], BF16, tag="ln2b_row")
            nc.sync.dma_start(out=ln2b_row[:], in_=ln2b[0:1, :])
            b2_row = cp.tile([1, DM], BF16, tag="b2_row")
            nc.sync.dma_start(out=b2_row[:], in_=b2v[0:1, :])
            ln2w_s = cp.tile([128, DM], BF16, tag="ln2w_bc")
            nc.gpsimd.partition_broadcast(ln2w_s[:], ln2w_row[0:1, :])
            ln2b_s = cp.tile([128, DM], BF16, tag="ln2b_bc")
            nc.gpsimd.partition_broadcast(ln2b_s[:], ln2b_row[0:1, :])
            b2_s = cp.tile([128, DM], BF16, tag="b2_bc")
            nc.gpsimd.partition_broadcast(b2_s[:], b2_row[0:1, :])

            # ---- resident activations ----
            u_raw = cp.tile([128, 2, NT], BF16, tag="u_raw")   # u; later gated y
            dec = cp.tile([128, 2, NT], BF16, tag="dec")       # E (ladder) / delta
            du = cp.tile([128, 2, NT], BF16, tag="du")         # delta * u

            # ============ Phase LN1 (transposed) + AllGather, per batch ========
            for b in range(B):
                cs = slice(b * TB, (b + 1) * TB)
                with nc.named_scope(f"ln1_{b}"):
                    ps_mu = p5.tile([128, 512], F32, tag="ps5", name=f"ln_mu{b}")
                    ps_sq = p5.tile([128, 512], F32, tag="ps5", name=f"ln_sq{b}")
                    for k in range(8):
                        xk = xbp.tile([128, TB], BF16, tag="xb", name=f"xa{b}{k}")
                        nc.gpsimd.dma_start(out=xk[:],
                                            in_=xT_own[k * 128:(k + 1) * 128, cs])
                        nc.tensor.matmul(ps_mu[0:1, :TB], ones_col[:], xk[:, :],
                                         start=(k == 0), stop=(k == 7))
                        sq = stg.tile([128, TB], BF16, tag="sq", name=f"sq{b}{k}")
                        nc.vector.tensor_tensor(out=sq[:], in0=xk[:], in1=xk[:],
                                                op=OP.mult)
                        nc.tensor.matmul(ps_sq[0:1, :TB], ones_col[:], sq[:, :],
                                         start=(k == 0), stop=(k == 7))
                    mu = lnp.tile([1, TB], F32, tag="ln_sm", name=f"mu{b}")
                    nc.vector.tensor_scalar(out=mu[:], in0=ps_mu[0:1, :TB],
                                            scalar1=1.0 / DM, scalar2=None,
                                            op0=OP.mult)
                    vr = lnp.tile([1, TB], F32, tag="ln_sm", name=f"vr{b}")
                    nc.vector.tensor_scalar(out=vr[:], in0=ps_sq[0:1, :TB],
                                            scalar1=1.0 / DM, scalar2=None,
                                            op0=OP.mult)
                    tm = lnp.tile([1, TB], F32, tag="ln_sm", name=f"tm{b}")
                    nc.vector.tensor_tensor(out=tm[:], in0=mu[:], in1=mu[:],
                                            op=OP.mult)
                    nc.vector.tensor_tensor(out=vr[:], in0=vr[:], in1=tm[:],
                                            op=OP.subtract)
                    nc.scalar.activation(out=vr[:], in_=vr[:], func=AF.Sqrt,
                                         bias=eps_s[0:1, :], scale=1.0)
                    nc.vector.reciprocal(out=vr[:], in_=vr[:])   # rstd
                    nc.vector.tensor_tensor(out=tm[:], in0=mu[:], in1=vr[:],
                                            op=OP.mult)          # mu*rstd
                    rstd_bf = lnp.tile([1, TB], BF16, tag="ln_smb",
                                       name=f"rstd_bf{b}")
                    nc.vector.tensor_copy(out=rstd_bf[:], in_=vr[:])
                    mrow_bf = lnp.tile([1, TB], BF16, tag="ln_smb",
                                       name=f"mrow_bf{b}")
                    nc.vector.tensor_copy(out=mrow_bf[:], in_=tm[:])
                    rstd_bc = lnp.tile([128, TB], BF16, tag="ln_bc",
                                       name=f"rstd_bc{b}")
                    nc.gpsimd.partition_broadcast(rstd_bc[:], rstd_bf[0:1, :])
                    mrow_bc = lnp.tile([128, TB], BF16, tag="ln_bc",
                                       name=f"mrow_bc{b}")
                    nc.gpsimd.partition_broadcast(mrow_bc[:], mrow_bf[0:1, :])
                    for k in range(8):
                        xk2 = xbp.tile([128, TB], BF16, tag="xb",
                                       name=f"xr{b}{k}")
                        nc.gpsimd.dma_start(
                            out=xk2[:], in_=xT_own[k * 128:(k + 1) * 128, cs])
                        t1 = stg.tile([128, TB], BF16, tag="sq", name=f"t1{b}{k}")
                        nc.vector.tensor_tensor(out=t1[:], in0=xk2[:],
                                                in1=rstd_bc[:], op=OP.mult)
                        nc.vector.tensor_tensor(out=t1[:], in0=t1[:],
                                                in1=mrow_bc[:], op=OP.subtract)
                        nc.vector.tensor_scalar(out=t1[:], in0=t1[:],
                                                scalar1=ln1wT_s[:, k:k + 1],
                                                scalar2=ln1bT_s[:, k:k + 1],
                                                op0=OP.mult, op1=OP.add)
                        nc.sync.dma_start(out=xnT_b[b][:, k, :], in_=t1[:])
                nc.gpsimd.collective_compute(
                    "AllGather", OP.bypass, replica_groups=GROUPS,
                    ins=[xnT_b[b][:, :, :]], outs=[xnT_all[b][:, :, :]])

            # ============ helper: in_proj for one batch ============
            def inproj(b):
                with nc.named_scope(f"inproj{b}"):
                    for src in range(NCORES):
                        xt = inp.tile([128, 8, TB], BF16, tag="xt")
                        nc.sync.dma_start(
                            out=xt[:],
                            in_=xnT_all[b][src * 128:(src + 1) * 128, :, :])
                        g0 = b * L + src * TB
                        psm = [p10.tile([128, 512], F32, tag="p10",
                                        name=f"ip{b}{src}{m}")
                               for m in range(4)]
                        for k in range(8):
                            for m in range(4):
                                nc.tensor.matmul(
                                    psm[m][:, :TB],
                                    w_in_s[:, k, m * 128:(m + 1) * 128],
                                    xt[:, k, :], start=(k == 0), stop=(k == 7))
                        for m in range(2):
                            nc.scalar.copy(out=u_raw[:, m, g0:g0 + TB],
                                           in_=psm[m][:, :TB])
                        for m in range(2):
                            zt = stg.tile([128, TB], BF16, tag="zt")
                            nc.scalar.copy(out=zt[:], in_=psm[2 + m][:, :TB])
                            nc.gpsimd.dma_start(
                                out=z_d[:, m, g0:g0 + TB], in_=zt[:])

            # ============ helper: conv + silu for one batch ============
            def conv(b):
                with nc.named_scope(f"conv{b}"):
                    for cc in range(2):
                        g0 = b * L
                        u_sl = u_raw[:, cc, g0:g0 + L]
                        cv = cvp.tile([128, L], BF16, tag="cv", name=f"cv{b}{cc}")
                        nc.vector.tensor_scalar(
                            out=cv[:], in0=u_sl, scalar1=conv_w_s[:, cc, 3:4],
                            scalar2=conv_b_s[:, cc:cc + 1], op0=OP.mult, op1=OP.add)
                        for k, sh in ((2, 1), (1, 2), (0, 3)):
                            nc.vector.scalar_tensor_tensor(
                                out=cv[:, sh:], in0=u_raw[:, cc, g0:g0 + L - sh],
                                scalar=conv_w_s[:, cc, k:k + 1], in1=cv[:, sh:],
                                op0=OP.mult, op1=OP.add)
                        nc.scalar.activation(out=u_sl, in_=cv[:], func=AF.Silu)

            # ============ helper: x_proj partial + AllReduce ============
            def xproj(b):
                with nc.named_scope(f"xproj{b}"):
                    for tc4 in range(4):
                        g0 = b * L + tc4 * 512
                        ps = p5.tile([128, 512], F32, tag="ps5",
                                     name=f"xp{b}{tc4}")
                        for cc in range(2):
                            nc.tensor.matmul(ps[:96, :], w_xp_s[:, cc, :],
                                             u_raw[:, cc, g0:g0 + 512],
                                             start=(cc == 0), stop=(cc == 1))
                        xd = stg.tile([96, 512], F32, tag="xd")
                        nc.scalar.copy(out=xd[:], in_=ps[:96, :])
                        nc.sync.dma_start(
                            out=xdbl_pt[b][:, tc4 * 512:(tc4 + 1) * 512],
                            in_=xd[:])
                nc.gpsimd.collective_compute(
                    "AllReduce", OP.add, replica_groups=GROUPS,
                    ins=[xdbl_pt[b][:, :]], outs=[xdbl_fl[b][:, :]])

            # ============ helper: dt_proj -> E/delta + du for one batch ========
            def dtproj(b):
                with nc.named_scope(f"dt{b}"):
                    dt_bf = rp.tile([64, L], BF16, tag="dtrow")
                    nc.gpsimd.dma_start(out=dt_bf[:], in_=xdbl_fl[b][0:64, :])
                    for cc in range(2):
                        for tc4 in range(4):
                            g0 = b * L + tc4 * 512
                            ps = p5.tile([128, 512], F32, tag="ps5",
                                         name=f"dt{b}{cc}{tc4}")
                            nc.tensor.matmul(ps[:], w_dt_s[:, cc, :],
                                             dt_bf[:, tc4 * 512:(tc4 + 1) * 512],
                                             start=True, stop=True)
                            # softplus(z) = ln(1 + e^z); exp/ln share one table
                            ex = stg.tile([128, 512], BF16, tag="dl",
                                          name=f"ex{b}{cc}{tc4}")
                            nc.scalar.activation(
                                out=ex[:], in_=ps[:], func=AF.Exp,
                                bias=dt_b_s[:, cc:cc + 1], scale=1.0)
                            if ladder:
                                dl = stg.tile([128, 512], BF16, tag="dl",
                                              name=f"dl{b}{cc}{tc4}")
                                nc.scalar.activation(
                                    out=dl[:], in_=ex[:], func=AF.Ln,
                                    bias=1.0, scale=1.0)
                                nc.gpsimd.tensor_tensor(
                                    out=du[:, cc, g0:g0 + 512], in0=dl[:],
                                    in1=u_raw[:, cc, g0:g0 + 512], op=OP.mult)
                                # E = exp(-delta)
                                nc.scalar.activation(
                                    out=dec[:, cc, g0:g0 + 512], in_=dl[:],
                                    func=AF.Exp, scale=-1.0)
                            else:
                                nc.scalar.activation(
                                    out=dec[:, cc, g0:g0 + 512], in_=ex[:],
                                    func=AF.Ln, bias=1.0, scale=1.0)
                                nc.gpsimd.tensor_tensor(
                                    out=du[:, cc, g0:g0 + 512],
                                    in0=dec[:, cc, g0:g0 + 512],
                                    in1=u_raw[:, cc, g0:g0 + 512], op=OP.mult)

            # ============ helper: scan for one batch (emits per-n pieces) ======
            def scan(b, inject=None):
                g0 = b * L
                with nc.named_scope(f"rows{b}"):
                    bcrows = rp.tile([32, L], BF16, tag="bcrow", name=f"bcr{b}")
                    nc.gpsimd.dma_start(out=bcrows[:], in_=xdbl_fl[b][64:96, :])
                accP, accV, a_prev = {}, {}, {}
                with nc.named_scope(f"scan{b}"):
                    for cc in range(2):
                        accP[cc] = trp.tile([128, L], BF16, tag="tr",
                                            name=f"aP{b}{cc}")
                        nc.vector.tensor_scalar(out=accP[cc][:],
                                                in0=u_raw[:, cc, g0:g0 + L],
                                                scalar1=dp_s[:, cc:cc + 1],
                                                scalar2=None, op0=OP.mult)
                    for n in range(DS):
                        # broadcast row n (B) / 16+n (C) of bcrows to all 128
                        # partitions via a selector matmul
                        b_bc = bcp.tile([128, L], BF16, tag="bc", name=f"b{b}_{n}")
                        c_bc = bcp.tile([128, L], BF16, tag="bc", name=f"c{b}_{n}")
                        for q in range(4):
                            sl = slice(q * 512, (q + 1) * 512)
                            psb = p5.tile([128, 512], F32, tag="ps5",
                                          name=f"bps{b}{n}{q}")
                            nc.tensor.matmul(psb[:],
                                             sel_s[:, n * 128:(n + 1) * 128],
                                             bcrows[:, sl],
                                             start=True, stop=True)
                            nc.scalar.copy(out=b_bc[:, sl], in_=psb[:])
                            psc = p5.tile([128, 512], F32, tag="ps5",
                                          name=f"cps{b}{n}{q}")
                            nc.tensor.matmul(psc[:],
                                             sel_s[:, (DS + n) * 128:
                                                   (DS + n + 1) * 128],
                                             bcrows[:, sl],
                                             start=True, stop=True)
                            nc.scalar.copy(out=c_bc[:, sl], in_=psc[:])
                        for cc in range(2):
                            if ladder:
                                if n == 0:
                                    a_n = dec[:, cc, g0:g0 + L]
                                else:
                                    a_t = lp.tile([128, L], BF16, tag="lad",
                                                  name=f"a{b}{cc}{n}")
                                    nc.vector.tensor_tensor(
                                        out=a_t[:], in0=a_prev[cc],
                                        in1=dec[:, cc, g0:g0 + L], op=OP.mult)
                                    a_n = a_t[:]
                                a_prev[cc] = a_n
                            else:
                                a_t = lp.tile([128, L], BF16, tag="lad",
                                              name=f"a{b}{cc}{n}")
                                nc.scalar.activation(
                                    out=a_t[:], in_=dec[:, cc, g0:g0 + L],
                                    func=AF.Exp, scale=a_s[:, cc, n:n + 1])
                                a_n = a_t[:]
                            bb = bbp.tile([128, L], BF16, tag="bb")
                            nc.vector.tensor_tensor(out=bb[:], in0=b_bc[:],
                                                    in1=du[:, cc, g0:g0 + L],
                                                    op=OP.mult)
                            h = hp.tile([128, L], BF16, tag="h")
                            nc.vector.tensor_tensor_scan(
                                out=h[:], data0=a_n, data1=bb[:], initial=0.0,
                                op0=OP.mult, op1=OP.add)
                            p = ppr.tile([128, L], BF16, tag="p",
                                         name=f"p{b}{cc}{n}")
                            nc.vector.tensor_tensor(out=p[:], in0=h[:], in1=c_bc[:],
                                                    op=OP.mult)
                            if n % 2 == 0:
                                nc.vector.tensor_tensor(out=accP[cc][:],
                                                        in0=accP[cc][:],
                                                        in1=p[:], op=OP.add)
                            elif n == 1:
                                accV[cc] = u_raw[:, cc, g0:g0 + L]
                                nc.vector.tensor_copy(out=accV[cc], in_=p[:])
                            else:
                                nc.vector.tensor_tensor(out=accV[cc],
                                                        in0=accV[cc],
                                                        in1=p[:], op=OP.add)
                        if inject and n in inject:
                            inject[n]()
                    # gate: y = (accP + accV) * silu(z)
                    for cc in range(2):
                        nc.vector.tensor_tensor(out=accV[cc], in0=accP[cc][:],
                                                in1=accV[cc], op=OP.add)
                        zf = cvp.tile([128, L], BF16, tag="cv", name=f"z{b}{cc}")
                        nc.sync.dma_start(out=zf[:], in_=z_d[:, cc, g0:g0 + L])
                        nc.scalar.activation(out=zf[:], in_=zf[:], func=AF.Silu)
                        nc.vector.tensor_tensor(out=u_raw[:, cc, g0:g0 + L],
                                                in0=accV[cc], in1=zf[:],
                                                op=OP.mult)

            # ============ helper: out_proj + RS for one batch ============
            def outproj(b):
                # token block tt -> slot row (tt%2)*1024 + (tt//2)*128 so each
                # 1024-row half scatters cores' own half-tokens; evens first,
                # then RS-A fires while the odd blocks compute.
                with nc.named_scope(f"outp{b}"):
                    for half in range(2):
                        for tt in range(half, 16, 2):
                            t0 = tt * 128
                            g0 = b * L + t0
                            r0 = half * 1024 + (tt // 2) * 128
                            yp = obp.tile([128, DM], BF16, tag="yp")
                            for f in range(2):
                                ps = p5.tile([128, 512], F32, tag="ps5",
                                             name=f"op{b}{tt}{f}")
                                for cc in range(2):
                                    nc.tensor.matmul(
                                        ps[:], u_raw[:, cc, g0:g0 + 128],
                                        w_out_s[:, cc, f * 512:(f + 1) * 512],
                                        start=(cc == 0), stop=(cc == 1))
                                nc.scalar.copy(out=yp[:, f * 512:(f + 1) * 512],
                                               in_=ps[:])
                            nc.sync.dma_start(out=yp_b[b][r0:r0 + 128, :],
                                              in_=yp[:])
                        nc.gpsimd.collective_compute(
                            "ReduceScatter", OP.add, replica_groups=GROUPS,
                            ins=[yp_b[b][half * 1024:(half + 1) * 1024, :]],
                            outs=[mamba_b[b][half * 128:(half + 1) * 128, :]])

            # ============ FFN split into parts for interleaving ============
            FFN = {}

            def ffn_part0(b):
                with nc.named_scope(f"ffnA{b}"):
                    FFN[b] = dict(
                        xn2T=fxp.tile([128, 8, TB], BF16, tag="xn2T",
                                      name=f"xn2T{b}"),
                        x2t={},
                        ps_o=[p10.tile([128, 512], F32, tag="p10",
                                       name=f"po{b}{t_}")
                              for t_ in range(4)])
                    xn2T = FFN[b]["xn2T"]
                    for tt in range(2):
                        r0 = tt * 128
                        xt = x2p.tile([128, DM], F32, tag="x2", name=f"x2{b}{tt}")
                        nc.sync.dma_start(
                            out=xt[:],
                            in_=x_own[b * TB + r0:b * TB + r0 + 128, :])
                        mo = stg.tile([128, DM], BF16, tag="mo")
                        nc.sync.dma_start(out=mo[:], in_=mamba_b[b][r0:r0 + 128, :])
                        nc.vector.tensor_tensor(out=xt[:], in0=xt[:], in1=mo[:],
                                                op=OP.add)
                        stats = lnp.tile([128, 2, 6], F32, tag="ln2st")
                        for s_ in range(2):
                            nc.vector.bn_stats(out=stats[:, s_, :],
                                               in_=xt[:, s_ * 512:(s_ + 1) * 512])
                        mv = lnp.tile([128, 2], F32, tag="ln2mv")
                        nc.vector.bn_aggr(out=mv[:], in_=stats[:])
                        rstd = lnp.tile([128, 1], F32, tag="ln2rs")
                        nc.scalar.activation(out=rstd[:], in_=mv[:, 1:2],
                                             func=AF.Sqrt, bias=eps_s[:],
                                             scale=1.0)
                        nc.vector.reciprocal(out=rstd[:], in_=rstd[:])
                        xn2 = stg.tile([128, DM], BF16, tag="xn2")
                        nc.vector.tensor_scalar(out=xn2[:], in0=xt[:],
                                                scalar1=mv[:, 0:1],
                                                scalar2=rstd[:],
                                                op0=OP.subtract, op1=OP.mult)
                        nc.vector.tensor_tensor(out=xn2[:], in0=xn2[:],
                                                in1=ln2w_s[:], op=OP.mult)
                        nc.vector.tensor_tensor(out=xn2[:], in0=xn2[:],
                                                in1=ln2b_s[:], op=OP.add)
                        nc.vector.tensor_tensor(out=xt[:], in0=xt[:], in1=b2_s[:],
                                                op=OP.add)
                        for k in range(8):
                            pst = ptr.tile([128, 128], BF16, tag="pstr")
                            nc.tensor.transpose(pst[:],
                                                xn2[:, k * 128:(k + 1) * 128],
                                                ident_s[:])
                            nc.scalar.copy(out=xn2T[:, k, r0:r0 + 128], in_=pst[:])
                        FFN[b]["x2t"][tt] = xt

            def ffn_part1(b, m0, m1):
                st = FFN[b]
                with nc.named_scope(f"ffnB{b}"):
                    for m in range(m0, m1):
                        w1_t = fwp.tile([128, 8, 128], BF16, tag="w1t")
                        nc.sync.dma_start(out=w1_t[:], in_=w1t[m, :, :, :])
                        ps1 = p5.tile([128, 512], F32, tag="ps5",
                                      name=f"f1{b}{m}")
                        for k in range(8):
                            nc.tensor.matmul(ps1[:, :TB], w1_t[:, k, :],
                                             st["xn2T"][:, k, :],
                                             start=(k == 0), stop=(k == 7))
                        h1 = fhp.tile([128, TB], BF16, tag="h1", name=f"h1{b}{m}")
                        nc.scalar.activation(out=h1[:], in_=ps1[:, :TB],
                                             func=AF.Relu, bias=b1_s[:, m:m + 1],
                                             scale=1.0)
                        w2_t = fwp.tile([128, DM], BF16, tag="w2t")
                        nc.sync.dma_start(out=w2_t[:],
                                          in_=w2t[m * 128:(m + 1) * 128, :])
                        for t_ in range(2):
                            for f in range(2):
                                nc.tensor.matmul(
                                    st["ps_o"][t_ * 2 + f][:],
                                    h1[:, t_ * 128:(t_ + 1) * 128],
                                    w2_t[:, f * 512:(f + 1) * 512],
                                    start=(m == 0), stop=(m == 31))

            def ffn_part2(b):
                st = FFN[b]
                with nc.named_scope(f"ffnC{b}"):
                    for t_ in range(2):
                        for f in range(2):
                            o = obp.tile([128, 512], F32, tag="ob",
                                         name=f"o{b}{t_}{f}")
                            nc.vector.tensor_tensor(
                                out=o[:],
                                in0=st["ps_o"][t_ * 2 + f][:],
                                in1=st["x2t"][t_][:, f * 512:(f + 1) * 512],
                                op=OP.add)
                            nc.sync.dma_start(
                                out=out_p[b * TB + t_ * 128:
                                          b * TB + (t_ + 1) * 128,
                                          f * 512:(f + 1) * 512],
                                in_=o[:])

            # ================= emission schedule =================
            inproj(0)
            conv(0)
            xproj(0)        # + AR0
            dtproj(0)
            inproj(1)
            conv(1)
            xproj(1)        # + AR1
            scan(0, inject={2: lambda: dtproj(1)})
            outproj(0)      # + RS0
            scan(1, inject={
                1: lambda: ffn_part0(0),
                2: lambda: ffn_part1(0, 0, 4),
                3: lambda: ffn_part1(0, 4, 8),
                4: lambda: ffn_part1(0, 8, 12),
                5: lambda: ffn_part1(0, 12, 16),
                6: lambda: ffn_part1(0, 16, 20),
                7: lambda: ffn_part1(0, 20, 24),
                8: lambda: ffn_part1(0, 24, 28),
                9: lambda: ffn_part1(0, 28, 32),
                10: lambda: ffn_part2(0),
            })
            outproj(1)      # + RS1
            ffn_part0(1)
            ffn_part1(1, 0, 32)
            ffn_part2(1)

    nc.finalize()
    _CACHE[key] = nc
    return nc


def _make_sel():
    m = np.zeros((32, 32 * 128), np.float32)
    for n in range(32):
        m[n, n * 128:(n + 1) * 128] = 1.0
    return m


def _prep_inputs(inputs):
    bf = ml_dtypes.bfloat16
    x = np.asarray(inputs["x"], np.float32)
    in_w = np.asarray(inputs["in_proj_w"], np.float32)
    conv_w = np.asarray(inputs["conv_w"], np.float32).reshape(DI, DC)
    conv_b = np.asarray(inputs["conv_b"], np.float32)
    xp_w = np.asarray(inputs["x_proj_w"], np.float32)
    dt_w = np.asarray(inputs["dt_proj_w"], np.float32)
    dt_bias = np.asarray(inputs["dt_proj_b"], np.float32)
    A = -np.exp(np.asarray(inputs["A_log"], np.float32))
    Dp = np.asarray(inputs["Dp"], np.float32)
    out_w = np.asarray(inputs["out_proj_w"], np.float32)
    w1 = np.asarray(inputs["ffn_w1"], np.float32)
    b1 = np.asarray(inputs["ffn_b1"], np.float32)
    w2 = np.asarray(inputs["ffn_w2"], np.float32)
    b2 = np.asarray(inputs["ffn_b2"], np.float32)

    ladder = bool(np.allclose(
        A, -np.arange(1, DS + 1, dtype=np.float32)[None, :], atol=1e-4))

    common = {
        "w1t": np.ascontiguousarray(
            w1.T.reshape(8, 128, 32, 128).transpose(2, 1, 0, 3)).astype(bf),
        "w2t": np.ascontiguousarray(w2.T).astype(bf),
        "b1m": np.ascontiguousarray(b1.reshape(32, 128).T).astype(np.float32),
        "b2v": b2.reshape(1, DM).astype(bf),
        "ln1wT": np.ascontiguousarray(
            np.asarray(inputs["ln1_w"], np.float32).reshape(8, 128).T),
        "ln1bT": np.ascontiguousarray(
            np.asarray(inputs["ln1_b"], np.float32).reshape(8, 128).T),
        "ln2w": inputs["ln2_w"].reshape(1, DM).astype(bf),
        "ln2b": inputs["ln2_b"].reshape(1, DM).astype(bf),
        "ident": np.eye(128, dtype=np.float32).astype(bf),
        "sel": _make_sel().astype(ml_dtypes.float8_e4m3),
    }

    def stripe2(v):  # [256] -> [128, 2]
        return np.ascontiguousarray(v.reshape(2, 128).T).astype(np.float32)

    in_maps = []
    for c in range(NCORES):
        ch0 = c * CH
        sl = slice(ch0, ch0 + CH)
        x_own = np.concatenate([x[b, c * TB:(c + 1) * TB, :] for b in range(B)],
                               axis=0)
        w_slice = np.concatenate([in_w[sl, :], in_w[DI + ch0:DI + ch0 + CH, :]],
                                 axis=0)
        w_in_t = np.ascontiguousarray(
            w_slice.T.reshape(8, 128, 512).transpose(1, 0, 2)).astype(bf)
        w_xp_t = np.ascontiguousarray(
            xp_w[:, sl].T.reshape(2, 128, 96).transpose(1, 0, 2)).astype(bf)
        w_dt_t = np.ascontiguousarray(
            dt_w[sl, :].T.reshape(64, 2, 128)).astype(bf)
        w_out_t = np.ascontiguousarray(
            out_w[:, sl].T.reshape(2, 128, 1024).transpose(1, 0, 2)).astype(bf)
        a_own = np.ascontiguousarray(
            A[sl].reshape(2, 128, DS).transpose(1, 0, 2)).astype(np.float32)
        cw = np.ascontiguousarray(
            conv_w[sl].reshape(2, 128, DC).transpose(1, 0, 2)).astype(np.float32)
        in_maps.append(dict(common,
            x_own=np.ascontiguousarray(x_own),
            xT_own=np.ascontiguousarray(x_own.T),
            w_in=w_in_t, w_xp=w_xp_t, w_dt=w_dt_t, w_out=w_out_t,
            conv_w=cw, conv_b=stripe2(conv_b[sl]),
            dt_b=stripe2(dt_bias[sl]), ndt_b=stripe2(-dt_bias[sl]),
            a_mat=a_own, dp_vec=stripe2(Dp[sl]),
        ))
    return in_maps, ladder


def _run(inputs, trace):
    in_maps, ladder = _prep_inputs(inputs)
    nc = build(ladder)
    res = run_bass_kernel_spmd(nc, in_maps, core_ids=list(range(NCORES)),
                               trace=trace)
    out = np.empty((B, L, DM), np.float32)
    for c in range(NCORES):
        o = res.results[c]["out"]
        for b in range(B):
            out[b, c * TB:(c + 1) * TB, :] = o[b * TB:(b + 1) * TB, :]
    return out, res


def kernel(**inputs):
    return _run(inputs, trace=False)[0]


def kernel_timed(**inputs):
    out, res = _run(inputs, trace=True)
    return out, res.exec_time_ns


def kernel_traced(**inputs):
    return _run(inputs, trace=True)
